# revision 63
# baseline (speedup 1.0000x reference)
"""Causal self-attention Bass kernel for 8 trn2 NeuronCores.

Problem: B=4, T=2048, D=1024, H=16 causal self-attention (qkv proj + attn + out proj).

Sharding: core c = 2*b + g handles batch b (=c//2) and head-group g (=c%2, 8 heads).
Per core:
  - qkv projection column-shard: q,k,v columns for its 8 heads only.
  - flash-style attention in transposed-score layout sT[tk, tq]; softmax denominator
    via an extra ones-column in the AV matmul (row 64 of the [65, 512] psum output).
  - output projection row-shard (w_proj rows for its head dims) -> partial [T, D].
  - pairwise ReduceScatter {2b, 2b+1} sums the two head-group partials and splits
    output rows t: even core -> rows [0,1024), odd -> [1024, 2048).
Host reassembles by stacking the two halves per batch.

Precision: matmuls run as float32r (1 cyc/row for N>=256). Q/K path additionally
uses bf16 storage for xT / w_qk (softmax is shift-robust: score errors are absolute
and scores are O(1)). Value path (v, attn weights, projections) stays f32/f32r.
b_v is folded into beta = b_proj(once per pair) + w_proj_shard.T @ b_v_shard since
softmax rows sum to 1.
"""

from contextlib import ExitStack

import ml_dtypes
import numpy as np

import concourse.bass as bass
import concourse.mybir as mybir
import concourse.tile as tile
from concourse import bacc
from concourse.bass_utils import run_bass_kernel_spmd

B, T, D, H = 4, 2048, 1024, 16
HD = D // H  # 64
NCORES = 8
P = 128
f32 = mybir.dt.float32
f32r = mybir.dt.float32r
bf16 = mybir.dt.bfloat16
EXP = mybir.ActivationFunctionType.Exp
LN = mybir.ActivationFunctionType.Ln

_CACHE = {}
LAST_RESULTS = None
_DEBUG_SINK = None


def _dbg(nc, name, ap):
    if _DEBUG_SINK is not None and name in _DEBUG_SINK:
        nc.sync.dma_start(_DEBUG_SINK[name].ap(), ap)


def _emit(nc, tc, x_d, wqk_d, wv_d, bqk_d, wproj_d, beta_d, selab_d, out_d):
    with ExitStack() as ctx:
        # ---------------- constants / persistent tiles ----------------
        const = ctx.enter_context(tc.tile_pool(name="const", bufs=1))
        bootc = ctx.enter_context(tc.tile_pool(name="boot", bufs=1))
        ident_f = bootc.tile([P, P], bf16, tag="ident_f")
        nc.gpsimd.memset(ident_f[:], 0.0)
        nc.gpsimd.affine_select(
            out=ident_f[:], in_=ident_f[:],
            compare_op=mybir.AluOpType.not_equal, fill=1.0,
            base=0, pattern=[[-1, P]], channel_multiplier=1,
        )
        ident = const.tile([P, P], f32r, tag="ident")
        nc.vector.tensor_copy(ident[:], ident_f[:])
        # (boot tiles stay resident; ~2.5KB)
        # triangle mask [128,128]: keep (1.0) iff f >= p
        mask_tri = const.tile([P, P], bf16, tag="mask_tri")
        nc.gpsimd.memset(mask_tri[:], 1.0)
        nc.gpsimd.affine_select(
            out=mask_tri[:], in_=mask_tri[:],
            compare_op=mybir.AluOpType.is_ge, fill=0.0,
            base=0, pattern=[[1, P]], channel_multiplier=-1,
        )
        bq = [const.tile([P, 1], f32, tag=f"bq{m}", name=f"bq{m}") for m in range(8)]
        beta_b = const.tile([P, D], bf16, tag="beta_b")
        # Per-core selectors for the denominator-reciprocal partition
        # broadcast: bc = sel.T @ rr puts rr row 0 on partitions 0-63, row 1
        # on 64-127. selE is the live pattern on even cores and all-zero on
        # odd cores (vice versa for selO) — this zeroes the exchange slot
        # that belongs to the peer, making the ReduceScatter sum a concat.
        selE = const.tile([2, P], f32r, tag="selE")
        selO = const.tile([2, P], f32r, tag="selO")

        def _load_small_consts():
            for m in range(8):
                nc.sync.dma_start(bq[m][:], bqk_d.ap()[m])
            nc.sync.dma_start(beta_b[0:1, :], beta_d.ap())
            nc.gpsimd.partition_broadcast(beta_b[:], beta_b[0:1, :], channels=P)
            nc.sync.dma_start(selE[:], selab_d.ap()[0])
            nc.sync.dma_start(selO[:], selab_d.ap()[1])
        # w_proj pool reserved here; its DMAs are emitted after phase 1 starts
        # so the x loads win the DMA queue.
        wpp = ctx.enter_context(tc.tile_pool(name="wpp", bufs=1))
        wproj_t = [wpp.tile([P, D], bf16, tag=f"wp{hp}", name=f"wp{hp}") for hp in range(8)]
        _dbg(nc, "beta_b", beta_b[:])

        # persistent activations
        xt_pool = ctx.enter_context(tc.tile_pool(name="xt", bufs=1))
        xT = [xt_pool.tile([P, T], bf16, tag=f"xT{k}", name=f"xT{k}") for k in range(8)]
        vv_pool = ctx.enter_context(tc.tile_pool(name="vv", bufs=1))
        vv = [vv_pool.tile([P, 520], bf16, tag=f"vv{i}", name=f"vv{i}") for i in range(16)]
        on_pool = ctx.enter_context(tc.tile_pool(name="outn", bufs=1))
        outN = [[on_pool.tile([P, 512], bf16, tag=f"outN{mp}J{J}", name=f"outN{mp}J{J}")
                 for J in range(4)] for mp in range(4)]
        zeros384 = const.tile([P, 384], bf16, tag="zeros384")
        nc.vector.memset(zeros384[:], 0.0)
        ones8 = const.tile([P, 8], bf16, tag="ones8")
        nc.vector.memset(ones8[:], 1.0)
        ones_row = const.tile([1, P], bf16, tag="ones_row")
        nc.vector.memset(ones_row[:], 1.0)
        ones_src = ones8[:].rearrange("p (mp h one) -> p mp h one", mp=4, h=2)
        for i in range(16):
            dst = vv[i][:].rearrange("p (mp h d) -> p mp h d", mp=4, h=2)
            nc.vector.tensor_copy(dst[:, :, :, 64:65], ones_src[:, :, :, :])

        # Head-half exchange buffers, chunked so only the last (single
        # head-pair) chunk is tail-exposed. In-tile rows per chunk (L pairs):
        # [shard s (T-col half) x [even-core slot (128L), odd-core slot
        # (128L)]]; each core writes BOTH slots, with the peer's slot zeroed
        # via selE/selO, so the pairwise ReduceScatter sum concatenates the
        # two cores' heads. Out rows arrive in a fixed global order on both
        # cores; wproj rows are host-permuted to match.
        CHUNKS = [(0, 1), (2,), (3,)]
        CHUNK_OF = {mp: ci for ci, mps in enumerate(CHUNKS) for mp in mps}
        dram = ctx.enter_context(tc.tile_pool(name="dram", bufs=1, space="DRAM"))
        a2a_in = [
            dram.tile([512 * len(mps), 1024], bf16, tag=f"a2a_in{c}",
                      name=f"a2a_in{c}")
            for c, mps in enumerate(CHUNKS)
        ]
        a2a_out = [
            dram.tile([256 * len(mps), 1024], bf16, tag=f"a2a_out{c}",
                      name=f"a2a_out{c}")
            for c, mps in enumerate(CHUNKS)
        ]

        # qk projection state (filled incrementally, interleaved into the
        # ACT-bound attention loop to keep the PE warm)
        qkt_pool = ctx.enter_context(tc.tile_pool(name="qkt", bufs=1))
        qkT = [qkt_pool.tile([P, T], bf16, tag=f"qkT{m}", name=f"qkT{m}") for m in range(8)]
        wqkp = ctx.enter_context(tc.tile_pool(name="wqk", bufs=2))
        wq_tiles = {}

        def qk_load(m):
            tiles = []
            for k in range(8):
                wt = wqkp.tile([P, P], bf16, tag=f"wqkt{k}", name=f"wqkt{k}")
                nc.sync.dma_start(
                    wt[:], wqk_d.ap()[k * P : (k + 1) * P, m * P : (m + 1) * P]
                )
                tiles.append(wt)
            wq_tiles[m] = tiles

        def qk_emit(pool, m, n):
            ps = pool.tile([P, 512], f32, tag="qkp", name="qkp")
            for k in range(8):
                nc.tensor.matmul(
                    ps[:], wq_tiles[m][k][:],
                    xT[k][:, n * 512 : (n + 1) * 512],
                    start=(k == 0), stop=(k == 7),
                )
            nc.vector.tensor_scalar_add(
                qkT[m][:, n * 512 : (n + 1) * 512], ps[:], bq[m][:]
            )

        # ---------------- phase 1: load x, transpose, compute v ----------------
        with ExitStack() as p1:
            xload = p1.enter_context(tc.tile_pool(name="xload", bufs=5))
            wvp = p1.enter_context(tc.tile_pool(name="wv", bufs=1))
            tpps = p1.enter_context(tc.tile_pool(name="tpps", bufs=2, space="PSUM"))
            vps = p1.enter_context(tc.tile_pool(name="vps", bufs=2, space="PSUM"))
            wv_t = [wvp.tile([P, 512], bf16, tag=f"wvt{k}", name=f"wvt{k}") for k in range(8)]
            for qq in range(4):  # t-quarters
                xi = []
                for ii in range(4):
                    xt_ = xload.tile([P, D], f32r, tag="x")
                    r0 = (qq * 4 + ii) * P
                    nc.sync.dma_start(xt_[:], x_d.ap()[r0 : r0 + P, :])
                    xi.append(xt_)
                if qq == 0:
                    # weight loads queue after the first x tiles
                    for k in range(8):
                        nc.sync.dma_start(wv_t[k][:], wv_d.ap()[k * P : (k + 1) * P, :])
                    qk_load(0)
                    qk_load(4)
                    _load_small_consts()
                for k in range(8):
                    tp = tpps.tile([P, 512], f32r, tag="tp")
                    for ii in range(4):
                        nc.tensor.transpose(
                            tp[:, ii * P : (ii + 1) * P],
                            xi[ii][:, k * P : (k + 1) * P],
                            ident[:],
                        )
                    nc.scalar.copy(xT[k][:, qq * 512 : (qq + 1) * 512], tp[:])
                # v for this quarter's 4 t-tiles (bf16 matmul off resident xT)
                for il in range(4):
                    i = qq * 4 + il
                    ps = vps.tile([P, 512], f32, tag="vp")
                    for k in range(8):
                        nc.tensor.matmul(
                            ps[:],
                            xT[k][:, (qq * 4 + il) * P : (qq * 4 + il + 1) * P],
                            wv_t[k][:],
                            start=(k == 0), stop=(k == 7),
                        )
                    # strided evict: psum [p, (mp h d)] d=64 -> vv [p, (mp h d65)]
                    src = ps[:].rearrange("p (mp h d) -> p mp h d", mp=4, h=2)
                    dst = vv[i][:].rearrange("p (mp h d) -> p mp h d", mp=4, h=2)
                    nc.vector.tensor_copy(dst[:, :, :, 0:64], src[:, :, :, :])
                # mp0's qk projection for this quarter rides the transpose/v
                # phase (xT quarter just landed)
                qk_emit(vps, 0, qq)
                qk_emit(vps, 4, qq)
            _dbg(nc, "xT0", xT[0][:])
            _dbg(nc, "vv0", vv[0][:])

        # ---------------- phase 2: per head-pair qkv + attention ----------------
        with ExitStack() as p2:
            atp = p2.enter_context(tc.tile_pool(name="atp", bufs=3))
            recip = p2.enter_context(tc.tile_pool(name="recip", bufs=4))
            shipp = p2.enter_context(tc.tile_pool(name="shipp", bufs=4))
            tmpb = p2.enter_context(tc.tile_pool(name="tmpb", bufs=2))
            dramDp = p2.enter_context(tc.tile_pool(name="dramDp", bufs=4, space="DRAM"))
            qkps = p2.enter_context(tc.tile_pool(name="qkps", bufs=1, space="PSUM"))
            stps = p2.enter_context(tc.tile_pool(name="stps", bufs=2, space="PSUM"))
            oups = p2.enter_context(tc.tile_pool(name="oups", bufs=1, space="PSUM"))
            auxps = p2.enter_context(tc.tile_pool(name="auxps", bufs=1, space="PSUM"))

            # deferred normalization stage-2: (mp, J, rr) emitted ~2 J-slots
            # later so the PE-broadcast matmul never waits on the reciprocal.
            pending = []

            def _norm_stage2(mp_, J_, rr_):
                c0 = (J_ % 2) * 512
                ci = CHUNK_OF[mp_]
                L = len(CHUNKS[ci])
                idx = mp_ - CHUNKS[ci][0]
                for sslot, sel in enumerate((selE, selO)):
                    bc = auxps.tile([P, 512], f32, tag="aux", name="bc")
                    nc.tensor.matmul(
                        bc[:], sel[:], rr_[:].bitcast(f32r), start=True, stop=True
                    )
                    tmp = shipp.tile([P, 512], bf16, tag="ship", name="ship")
                    nc.vector.tensor_mul(tmp[:], outN[mp_][J_][:], bc[:])
                    r0 = (J_ // 2) * 256 * L + sslot * 128 * L + idx * 128
                    nc.scalar.dma_start(
                        a2a_in[ci][r0 : r0 + P, c0 : c0 + 512], tmp[:]
                    )

            def _flush_pending(upto_slot):
                while pending and pending[0][0] <= upto_slot:
                    _, mp_, J_, rr_ = pending.pop(0)
                    _norm_stage2(mp_, J_, rr_)

            def _emit_exchange(ci):
                half = 256 * len(CHUNKS[ci])
                if globals().get("_NO_COLLECTIVE"):
                    nc.sync.dma_start(a2a_out[ci].opt(), a2a_in[ci][0:half, :])
                else:
                    nc.gpsimd.collective_compute(
                        "ReduceScatter", mybir.AluOpType.add,
                        replica_groups=[[0, 1], [2, 3], [4, 5], [6, 7]],
                        ins=[a2a_in[ci].opt()], outs=[a2a_out[ci].opt()],
                    )

            for mp in range(4):
                qs, ks = qkT[mp], qkT[4 + mp]
                for J in range(4):
                    _flush_pending(4 * mp + J - 1)
                    # interleave the NEXT head-pair's qk projection into this
                    # (ACT-bound) attention block: weights at J=0/2, two
                    # matmul groups per J.
                    if mp < 3:
                        if J == 0:
                            qk_load(mp + 1)
                        if J == 2:
                            qk_load(5 + mp)
                        qk_m = (mp + 1) if J < 2 else (5 + mp)
                        qk_ns = (2 * (J % 2), 2 * (J % 2) + 1)
                    if mp == 1 and J == 0:
                        # w_proj only needed at projection time; keep it off
                        # the phase-1 DMA queue
                        for hp in range(8):
                            nc.sync.dma_start(
                                wproj_t[hp][:],
                                wproj_d.ap()[hp * P : (hp + 1) * P, :],
                            )
                    if mp == 3 and J == 0:
                        # xT is dead once mp3's qk groups were emitted (end of
                        # mp2): prefetch the first two exchange chunks into the
                        # xT tiles while mp3's attention runs. On the gpsimd
                        # (SWDGE) queue: these wait on the collectives, and on
                        # the sync ring that wait would head-of-line-block the
                        # reciprocal chains behind it.
                        for h in range(6):
                            ci, r = (0, h) if h < 4 else (1, h - 4)
                            nc.gpsimd.dma_start(
                                xT[h][:, 0:1024],
                                a2a_out[ci][r * P : (r + 1) * P, :],
                            )
                    nj = 4 * J + 4
                    ouA = oups.tile([65, 512], f32, tag="ouA")
                    ouB = oups.tile([65, 512], f32, tag="ouB")
                    Js = slice(J * 512, (J + 1) * 512)
                    for j in range(nj):
                        sT = stps.tile([P, 1024], f32, tag="sT")
                        js = slice(j * P, (j + 1) * P)
                        nc.tensor.matmul(
                            sT[:, 0:512],
                            ks[0:64, js], qs[0:64, Js],
                            start=True, stop=True, tile_position=(0, 0),
                        )
                        nc.tensor.matmul(
                            sT[:, 512:1024],
                            ks[64:128, js], qs[64:128, Js],
                            start=True, stop=True, tile_position=(64, 0),
                        )
                        at = atp.tile([P, 1024], bf16, tag="at")
                        i = j - 4 * J
                        if i > 0:
                            c0 = 128 * i
                            src_v = sT[:].rearrange("p (h c) -> p h c", h=2)
                            dst_v = at[:].rearrange("p (h c) -> p h c", h=2)
                            nc.scalar.activation(
                                dst_v[:, :, c0:512], src_v[:, :, c0:512],
                                EXP, bias=0.0, scale=0.125,
                            )
                        else:
                            nc.scalar.activation(at[:], sT[:], EXP, bias=0.0, scale=0.125)
                        if i >= 0:
                            # diagonal-straddling block: zero cols < 128i, apply
                            # the triangle on cols [128i, 128i+128)
                            for h0 in (0, 512):
                                c0 = h0 + 128 * i
                                if i > 0:
                                    nc.vector.tensor_copy(
                                        at[:, h0 : h0 + 128 * i],
                                        zeros384[:, 0 : 128 * i],
                                    )
                                nc.vector.tensor_mul(
                                    at[:, c0 : c0 + 128],
                                    at[:, c0 : c0 + 128], mask_tri[:],
                                )
                        if mp == 0 and J == 0 and j == 0:
                            _dbg(nc, "at000", at[:])
                        nc.tensor.matmul(
                            ouA[:], vv[j][:, 130 * mp : 130 * mp + 65],
                            at[:, 0:512],
                            start=(j == 0), stop=(j == nj - 1),
                        )
                        nc.tensor.matmul(
                            ouB[:], vv[j][:, 130 * mp + 65 : 130 * mp + 130],
                            at[:, 512:1024],
                            start=(j == 0), stop=(j == nj - 1),
                        )
                        if mp < 3 and (j == nj // 2 - 1 or j == nj - 1):
                            qk_emit(qkps, qk_m, qk_ns[0 if j == nj // 2 - 1 else 1])
                    # normalize by softmax denominator (psum row 64) and evict
                    if mp == 0 and J == 0 and _DEBUG_SINK is not None:
                        for _nm, _ou in (("ouA00", ouA), ("ouB00", ouB)):
                            if _nm in _DEBUG_SINK:
                                _dt = atp.tile([65, 512], f32, tag=f"dbg{_nm}", name=f"dbg{_nm}")
                                nc.vector.tensor_copy(_dt[:], _ou[:])
                                nc.sync.dma_start(_DEBUG_SINK[_nm].ap(), _dt[:])
                    # Normalization stage 1: raw-evict o as bf16 (frees the
                    # psum banks fast), pull the denominator rows out, repack
                    # them onto 128 partitions via tiny SBUF-SBUF DMAs, and
                    # fast-reciprocal there. Stage 2 (PE-broadcast + multiply
                    # + ship to the exchange buffer) is deferred two J-slots
                    # so nothing ever waits on this chain.
                    dd = recip.tile([1, 1024], f32, tag="dd", name="dd")
                    rr = recip.tile([2, 512], f32, tag="rr", name="rr")
                    tb = tmpb.tile([64, 512], bf16, tag="tb")
                    nc.vector.tensor_copy(dd[0:1, 0:512], ouA[64:65, :])
                    nc.vector.tensor_copy(outN[mp][J][0:64, :], ouA[0:64, :])
                    nc.vector.tensor_copy(dd[0:1, 512:1024], ouB[64:65, :])
                    nc.vector.tensor_copy(tb[:], ouB[0:64, :])
                    # head B lives on partitions 64-127: DVE can't cross
                    # partitions, so DMA-shift the block up (ACT HWDGE ring;
                    # the sync ring carries the bulk loads).
                    nc.scalar.dma_start(outN[mp][J][64:128, :], tb[:])
                    # reciprocal on the single denominator row, then a DRAM
                    # hop to split it across partitions 0-1 for the broadcast
                    nc.vector.reciprocal_approx_fast(dd[:], dd[:])
                    dramD = dramDp.tile([2, 512], f32, tag="dramD", name="dramD")
                    nc.sync.dma_start(dramD[:].rearrange("a c -> (a c)").unsqueeze(0), dd[:])
                    nc.sync.dma_start(rr[:], dramD[:])
                    pending.append((4 * mp + J, mp, J, rr))
                    if mp == 0 and J == 0:
                        _dbg(nc, "outNraw00", outN[0][0][:])
                        _dbg(nc, "dd00", dd[:])

                if mp == CHUNKS[0][-1]:
                    _flush_pending(4 * mp + 3)
                    _emit_exchange(0)
                elif mp == CHUNKS[1][-1]:
                    _flush_pending(4 * mp + 3)
                    _emit_exchange(1)
            _flush_pending(15)
            _emit_exchange(2)
            _dbg(nc, "qkT0", qkT[0][:])
            _dbg(nc, "qkT4", qkT[4][:])

            # ---- output projection over my T-half, contracting all 16 heads.
            # po rows arrive in group-rank order == global head order on both
            # cores; wproj rows are host-permuted to match.
            # exchange landing buffers reuse the (dead) xT tiles; chunks 0-1
            # were prefetched at mp3 J0, only chunk 2 lands here.
            finp = p2.enter_context(tc.tile_pool(name="finp", bufs=6))
            po = list(xT)
            for h in (6, 7):
                nc.gpsimd.dma_start(
                    xT[h][:, 0:1024], a2a_out[2][(h - 6) * P : (h - 5) * P, :]
                )
            _dbg(nc, "po0", po[0][:, 0:1024])
            for i in range(8):
                for n in range(2):
                    u = i * 2 + n
                    pool_, tag_ = (qkps, "qkp") if u % 2 == 0 else (auxps, "aux")
                    ps = pool_.tile([P, 512], f32, tag=tag_, name="fp")
                    for hp in range(8):
                        nc.tensor.matmul(
                            ps[:],
                            po[hp][:, i * P : (i + 1) * P],
                            wproj_t[hp][:, n * 512 : (n + 1) * 512],
                            start=(hp == 0), stop=False,
                        )
                    # beta folded in as a rank-1 matmul so the eviction is a
                    # plain copy
                    nc.tensor.matmul(
                        ps[:], ones_row[:], beta_b[0:1, n * 512 : (n + 1) * 512],
                        start=False, stop=True,
                    )
                    fin = finp.tile([P, 512], f32, tag="fin", name="fin")
                    nc.vector.tensor_copy(fin[:], ps[:])
                    eng = nc.sync if u % 2 == 0 else nc.scalar
                    eng.dma_start(
                        out_d.ap()[i * P : (i + 1) * P, n * 512 : (n + 1) * 512], fin[:]
                    )


def _build():
    if "nc" in _CACHE:
        return _CACHE["nc"]
    global _DEBUG_SINK
    nc = bacc.Bacc("TRN2", target_bir_lowering=False, debug=False, num_devices=NCORES)
    spec = globals().get("_DEBUG_SINK_SPEC")
    if spec:
        _DEBUG_SINK = {
            name: nc.dram_tensor(name, list(shape), dt, kind="ExternalOutput")
            for name, (shape, dt) in spec.items()
        }
    x_d = nc.dram_tensor("x", [T, D], f32r, kind="ExternalInput")
    wqk_d = nc.dram_tensor("w_qk", [D, 1024], bf16, kind="ExternalInput")
    wv_d = nc.dram_tensor("w_v", [D, 512], bf16, kind="ExternalInput")
    bqk_d = nc.dram_tensor("b_qk", [8, P, 1], f32, kind="ExternalInput")
    wproj_d = nc.dram_tensor("w_proj", [D, D], bf16, kind="ExternalInput")
    beta_d = nc.dram_tensor("beta", [1, D], bf16, kind="ExternalInput")
    selab_d = nc.dram_tensor("selab", [2, 2, P], f32r, kind="ExternalInput")
    out_d = nc.dram_tensor("out", [T // 2, D], f32, kind="ExternalOutput")
    with tile.TileContext(nc) as tc:
        _emit(nc, tc, x_d, wqk_d, wv_d, bqk_d, wproj_d, beta_d, selab_d, out_d)
    nc.compile()
    _CACHE["nc"] = nc
    return nc


def make_in_maps(x, w_qkv, b_qkv, w_proj, b_proj):
    x = np.asarray(x, np.float32)
    w_qkv = np.asarray(w_qkv, np.float32)
    b_qkv = np.asarray(b_qkv, np.float32)
    w_proj = np.asarray(w_proj, np.float32)
    b_proj = np.asarray(b_proj, np.float32)
    # w_proj rows permuted into the exchange arrival order: for chunk c and
    # group rank gr, the head-pairs (mp, 4+mp) for mp in {2c, 2c+1} of core gr.
    # head-pair mp holds heads (2mp, 2mp+1) of the core's 8 (qkT/vv layout).
    # Chunk layout must match CHUNKS in _emit.
    perm = []
    for mps in ((0, 1), (2,), (3,)):
        for gr in range(2):
            for mp in mps:
                for m in (2 * mp, 2 * mp + 1):
                    h = gr * 8 + m
                    perm.extend(range(h * HD, (h + 1) * HD))
    wp_perm = np.ascontiguousarray(w_proj[perm, :]).astype(ml_dtypes.bfloat16)
    beta = (b_proj + w_proj.T @ b_qkv[2 * D :]).reshape(1, D).astype(ml_dtypes.bfloat16)
    # selector pattern: row 0 -> partitions 0-63 (head A), row 1 -> 64-127.
    selpat = np.zeros((2, P), np.float32)
    selpat[0, 0:64] = 1.0
    selpat[1, 64:128] = 1.0
    selz = np.zeros((2, P), np.float32)
    in_maps = []
    for c in range(NCORES):
        b, g = c // 2, c % 2
        qcols = slice(g * 512, (g + 1) * 512)
        kcols = slice(D + g * 512, D + (g + 1) * 512)
        vcols = slice(2 * D + g * 512, 2 * D + (g + 1) * 512)
        w_qk = np.concatenate([w_qkv[:, qcols], w_qkv[:, kcols]], axis=1)
        b_qk = np.concatenate([b_qkv[qcols], b_qkv[kcols]])
        in_maps.append({
            "x": np.ascontiguousarray(x[b]),
            "w_qk": np.ascontiguousarray(w_qk).astype(ml_dtypes.bfloat16),
            "w_v": np.ascontiguousarray(w_qkv[:, vcols]).astype(ml_dtypes.bfloat16),
            "b_qk": b_qk.reshape(8, P, 1),
            "w_proj": wp_perm,
            "beta": beta,
            "selab": np.stack([selpat, selz] if g == 0 else [selz, selpat]),
        })
    return in_maps


def kernel(x, w_qkv, b_qkv, w_proj, b_proj, trace=False, **run_kwargs):
    global LAST_RESULTS
    nc = _build()
    in_maps = make_in_maps(x, w_qkv, b_qkv, w_proj, b_proj)
    res = run_bass_kernel_spmd(
        nc, in_maps, core_ids=list(range(NCORES)), trace=trace, **run_kwargs
    )
    LAST_RESULTS = res
    out = np.empty((B, T, D), np.float32)
    for b in range(B):
        out[b, : T // 2] = res.results[2 * b]["out"]
        out[b, T // 2 :] = res.results[2 * b + 1]["out"]
    return out



# revision 64
# speedup vs baseline: 1.0304x; 1.0304x over previous
"""Causal self-attention Bass kernel for 8 trn2 NeuronCores.

Problem: B=4, T=2048, D=1024, H=16 causal self-attention (qkv proj + attn + out proj).

Sharding: core c = 2*b + g handles batch b (=c//2) and head-group g (=c%2, 8 heads).
Per core:
  - qkv projection column-shard: q,k,v columns for its 8 heads only. The q/k
    matmul groups for head-pair mp+1 are interleaved into the (ACT-bound)
    attention loop of head-pair mp so the PE stays dense and HAM-warm; mp0's
    ride the phase-1 transpose/v pipeline.
  - flash-style attention in transposed-score layout sT[tk, tq]; softmax
    denominator via an extra ones-column in the AV matmul (row 64 of the
    [65, 512] psum output). Normalization: fast-approx reciprocal of the
    denominator row, partition-broadcast via a tiny PE matmul (sel.T @ rr),
    deferred one J-slot so nothing waits on the chain.
  - instead of projecting partials and ReduceScattering [T, D] f32, the
    normalized per-head outputs o (bf16, 4x less data) are exchanged between
    the two cores of a batch with pairwise ReduceScatters: each core writes
    its o into both the even- and odd-core row slots of the exchange buffer,
    with the peer's slot zeroed via a per-core 0/1 selector, so the RS sum
    concatenates the heads. 3 chunks (mp01 / mp2 / mp3) so only the last
    1MB chunk is tail-exposed.
  - each core then projects its T-half contracting all 16 heads (w_proj rows
    host-permuted into exchange arrival order, beta folded in as a rank-1
    matmul), writing out rows [0,1024) (even core) / [1024,2048) (odd).
Host reassembles by stacking the two halves per batch.

Precision: matmuls bf16 (x, q/k, attn weights, v, o, w_proj) with f32 psum;
softmax is shift-robust and the 2e-2 rel-err budget absorbs bf16 rounding
(measured ~3.4e-3). b_v is folded into beta = b_proj + w_proj.T @ b_v since
softmax rows sum to 1.
"""

from contextlib import ExitStack

import ml_dtypes
import numpy as np

import concourse.bass as bass
import concourse.mybir as mybir
import concourse.tile as tile
from concourse import bacc
from concourse.bass_utils import run_bass_kernel_spmd

B, T, D, H = 4, 2048, 1024, 16
HD = D // H  # 64
NCORES = 8
P = 128
f32 = mybir.dt.float32
f32r = mybir.dt.float32r
bf16 = mybir.dt.bfloat16
EXP = mybir.ActivationFunctionType.Exp
LN = mybir.ActivationFunctionType.Ln

_CACHE = {}
LAST_RESULTS = None
_DEBUG_SINK = None


def _dbg(nc, name, ap):
    if _DEBUG_SINK is not None and name in _DEBUG_SINK:
        nc.sync.dma_start(_DEBUG_SINK[name].ap(), ap)


def _emit(nc, tc, x_d, wqk_d, wv_d, bqk_d, wproj_d, beta_d, selab_d, out_d):
    with ExitStack() as ctx:
        # ---------------- constants / persistent tiles ----------------
        const = ctx.enter_context(tc.tile_pool(name="const", bufs=1))
        bootc = ctx.enter_context(tc.tile_pool(name="boot", bufs=1))
        ident_f = bootc.tile([P, P], bf16, tag="ident_f")
        nc.gpsimd.memset(ident_f[:], 0.0)
        nc.gpsimd.affine_select(
            out=ident_f[:], in_=ident_f[:],
            compare_op=mybir.AluOpType.not_equal, fill=1.0,
            base=0, pattern=[[-1, P]], channel_multiplier=1,
        )
        ident = const.tile([P, P], f32r, tag="ident")
        nc.vector.tensor_copy(ident[:], ident_f[:])
        # (boot tiles stay resident; ~2.5KB)
        # triangle mask [128,128]: keep (1.0) iff f >= p
        mask_tri = const.tile([P, P], bf16, tag="mask_tri")
        nc.gpsimd.memset(mask_tri[:], 1.0)
        nc.gpsimd.affine_select(
            out=mask_tri[:], in_=mask_tri[:],
            compare_op=mybir.AluOpType.is_ge, fill=0.0,
            base=0, pattern=[[1, P]], channel_multiplier=-1,
        )
        bq = [const.tile([P, 1], f32, tag=f"bq{m}", name=f"bq{m}") for m in range(8)]
        beta_b = const.tile([P, D], bf16, tag="beta_b")
        # Per-core selectors for the denominator-reciprocal partition
        # broadcast: bc = sel.T @ rr puts rr row 0 on partitions 0-63, row 1
        # on 64-127. selE is the live pattern on even cores and all-zero on
        # odd cores (vice versa for selO) — this zeroes the exchange slot
        # that belongs to the peer, making the ReduceScatter sum a concat.
        selE = const.tile([2, P], f32r, tag="selE")
        selO = const.tile([2, P], f32r, tag="selO")

        def _load_small_consts():
            for m in range(8):
                nc.sync.dma_start(bq[m][:], bqk_d.ap()[m])
            nc.sync.dma_start(beta_b[0:1, :], beta_d.ap())
            nc.gpsimd.partition_broadcast(beta_b[:], beta_b[0:1, :], channels=P)
            nc.sync.dma_start(selE[:], selab_d.ap()[0])
            nc.sync.dma_start(selO[:], selab_d.ap()[1])
        # w_proj pool reserved here; its DMAs are emitted after phase 1 starts
        # so the x loads win the DMA queue.
        wpp = ctx.enter_context(tc.tile_pool(name="wpp", bufs=1))
        wproj_t = [wpp.tile([P, D], bf16, tag=f"wp{hp}", name=f"wp{hp}") for hp in range(8)]
        _dbg(nc, "beta_b", beta_b[:])

        # persistent activations
        xt_pool = ctx.enter_context(tc.tile_pool(name="xt", bufs=1))
        xT = [xt_pool.tile([P, T], bf16, tag=f"xT{k}", name=f"xT{k}") for k in range(8)]
        vv_pool = ctx.enter_context(tc.tile_pool(name="vv", bufs=1))
        vv = [vv_pool.tile([P, 520], bf16, tag=f"vv{i}", name=f"vv{i}") for i in range(16)]
        on_pool = ctx.enter_context(tc.tile_pool(name="outn", bufs=1))
        outN = [[on_pool.tile([P, 512], bf16, tag=f"outN{mp}J{J}", name=f"outN{mp}J{J}")
                 for J in range(4)] for mp in range(4)]
        zeros384 = const.tile([P, 384], bf16, tag="zeros384")
        nc.vector.memset(zeros384[:], 0.0)
        ones8 = const.tile([P, 8], bf16, tag="ones8")
        nc.vector.memset(ones8[:], 1.0)
        ones_row = const.tile([1, P], bf16, tag="ones_row")
        nc.vector.memset(ones_row[:], 1.0)
        ones_src = ones8[:].rearrange("p (mp h one) -> p mp h one", mp=4, h=2)
        for i in range(16):
            dst = vv[i][:].rearrange("p (mp h d) -> p mp h d", mp=4, h=2)
            nc.vector.tensor_copy(dst[:, :, :, 64:65], ones_src[:, :, :, :])

        # Head-half exchange buffers, chunked so only the last (single
        # head-pair) chunk is tail-exposed. In-tile rows per chunk (L pairs):
        # [shard s (T-col half) x [even-core slot (128L), odd-core slot
        # (128L)]]; each core writes BOTH slots, with the peer's slot zeroed
        # via selE/selO, so the pairwise ReduceScatter sum concatenates the
        # two cores' heads. Out rows arrive in a fixed global order on both
        # cores; wproj rows are host-permuted to match.
        CHUNKS = [(0, 1), (2,), (3,)]
        CHUNK_OF = {mp: ci for ci, mps in enumerate(CHUNKS) for mp in mps}
        dram = ctx.enter_context(tc.tile_pool(name="dram", bufs=1, space="DRAM"))
        a2a_in = [
            dram.tile([512 * len(mps), 1024], bf16, tag=f"a2a_in{c}",
                      name=f"a2a_in{c}")
            for c, mps in enumerate(CHUNKS)
        ]
        a2a_out = [
            dram.tile([256 * len(mps), 1024], bf16, tag=f"a2a_out{c}",
                      name=f"a2a_out{c}")
            for c, mps in enumerate(CHUNKS)
        ]

        # qk projection state (filled incrementally, interleaved into the
        # ACT-bound attention loop to keep the PE warm)
        qkt_pool = ctx.enter_context(tc.tile_pool(name="qkt", bufs=1))
        qkT = [qkt_pool.tile([P, T], bf16, tag=f"qkT{m}", name=f"qkT{m}") for m in range(8)]
        wqkp = ctx.enter_context(tc.tile_pool(name="wqk", bufs=2))
        wq_tiles = {}

        def qk_load(m):
            tiles = []
            for k in range(8):
                wt = wqkp.tile([P, P], bf16, tag=f"wqkt{k}", name=f"wqkt{k}")
                nc.sync.dma_start(
                    wt[:], wqk_d.ap()[k * P : (k + 1) * P, m * P : (m + 1) * P]
                )
                tiles.append(wt)
            wq_tiles[m] = tiles

        def qk_emit(pool, m, n):
            ps = pool.tile([P, 512], f32, tag="qkp", name="qkp")
            for k in range(8):
                nc.tensor.matmul(
                    ps[:], wq_tiles[m][k][:],
                    xT[k][:, n * 512 : (n + 1) * 512],
                    start=(k == 0), stop=(k == 7),
                )
            nc.vector.tensor_scalar_add(
                qkT[m][:, n * 512 : (n + 1) * 512], ps[:], bq[m][:]
            )

        # ---------------- phase 1: load x, transpose, compute v ----------------
        with ExitStack() as p1:
            xload = p1.enter_context(tc.tile_pool(name="xload", bufs=5))
            wvp = p1.enter_context(tc.tile_pool(name="wv", bufs=1))
            tpps = p1.enter_context(tc.tile_pool(name="tpps", bufs=2, space="PSUM"))
            vps = p1.enter_context(tc.tile_pool(name="vps", bufs=2, space="PSUM"))
            wv_t = [wvp.tile([P, 512], bf16, tag=f"wvt{k}", name=f"wvt{k}") for k in range(8)]
            for qq in range(4):  # t-quarters
                xi = []
                for ii in range(4):
                    xt_ = xload.tile([P, D], f32r, tag="x")
                    r0 = (qq * 4 + ii) * P
                    nc.sync.dma_start(xt_[:], x_d.ap()[r0 : r0 + P, :])
                    xi.append(xt_)
                if qq == 0:
                    # weight loads queue after the first x tiles
                    for k in range(8):
                        nc.sync.dma_start(wv_t[k][:], wv_d.ap()[k * P : (k + 1) * P, :])
                    qk_load(0)
                    qk_load(4)
                    _load_small_consts()
                for k in range(8):
                    tp = tpps.tile([P, 512], f32r, tag="tp")
                    for ii in range(4):
                        nc.tensor.transpose(
                            tp[:, ii * P : (ii + 1) * P],
                            xi[ii][:, k * P : (k + 1) * P],
                            ident[:],
                        )
                    nc.scalar.copy(xT[k][:, qq * 512 : (qq + 1) * 512], tp[:])
                # v for this quarter's 4 t-tiles (bf16 matmul off resident xT)
                for il in range(4):
                    i = qq * 4 + il
                    ps = vps.tile([P, 512], f32, tag="vp")
                    for k in range(8):
                        nc.tensor.matmul(
                            ps[:],
                            xT[k][:, (qq * 4 + il) * P : (qq * 4 + il + 1) * P],
                            wv_t[k][:],
                            start=(k == 0), stop=(k == 7),
                        )
                    # strided evict: psum [p, (mp h d)] d=64 -> vv [p, (mp h d65)]
                    src = ps[:].rearrange("p (mp h d) -> p mp h d", mp=4, h=2)
                    dst = vv[i][:].rearrange("p (mp h d) -> p mp h d", mp=4, h=2)
                    nc.vector.tensor_copy(dst[:, :, :, 0:64], src[:, :, :, :])
                # mp0's qk projection for this quarter rides the transpose/v
                # phase (xT quarter just landed)
                qk_emit(vps, 0, qq)
                qk_emit(vps, 4, qq)
            _dbg(nc, "xT0", xT[0][:])
            _dbg(nc, "vv0", vv[0][:])

        # ---------------- phase 2: per head-pair qkv + attention ----------------
        with ExitStack() as p2:
            atp = p2.enter_context(tc.tile_pool(name="atp", bufs=3))
            recip = p2.enter_context(tc.tile_pool(name="recip", bufs=4))
            shipp = p2.enter_context(tc.tile_pool(name="shipp", bufs=4))
            tmpb = p2.enter_context(tc.tile_pool(name="tmpb", bufs=2))
            dramDp = p2.enter_context(tc.tile_pool(name="dramDp", bufs=4, space="DRAM"))
            qkps = p2.enter_context(tc.tile_pool(name="qkps", bufs=1, space="PSUM"))
            stps = p2.enter_context(tc.tile_pool(name="stps", bufs=2, space="PSUM"))
            oups = p2.enter_context(tc.tile_pool(name="oups", bufs=1, space="PSUM"))
            auxps = p2.enter_context(tc.tile_pool(name="auxps", bufs=1, space="PSUM"))

            # deferred normalization stage-2: (mp, J, rr) emitted ~2 J-slots
            # later so the PE-broadcast matmul never waits on the reciprocal.
            pending = []

            def _norm_stage2(mp_, J_, rr_):
                c0 = (J_ % 2) * 512
                ci = CHUNK_OF[mp_]
                L = len(CHUNKS[ci])
                idx = mp_ - CHUNKS[ci][0]
                for sslot, sel in enumerate((selE, selO)):
                    bc = auxps.tile([P, 512], f32, tag="aux", name="bc")
                    nc.tensor.matmul(
                        bc[:], sel[:], rr_[:].bitcast(f32r), start=True, stop=True
                    )
                    tmp = shipp.tile([P, 512], bf16, tag="ship", name="ship")
                    nc.vector.tensor_mul(tmp[:], outN[mp_][J_][:], bc[:])
                    r0 = (J_ // 2) * 256 * L + sslot * 128 * L + idx * 128
                    nc.scalar.dma_start(
                        a2a_in[ci][r0 : r0 + P, c0 : c0 + 512], tmp[:]
                    )

            def _flush_pending(upto_slot):
                while pending and pending[0][0] <= upto_slot:
                    _, mp_, J_, rr_ = pending.pop(0)
                    _norm_stage2(mp_, J_, rr_)

            def _emit_exchange(ci):
                half = 256 * len(CHUNKS[ci])
                if globals().get("_NO_COLLECTIVE"):
                    nc.sync.dma_start(a2a_out[ci].opt(), a2a_in[ci][0:half, :])
                else:
                    nc.gpsimd.collective_compute(
                        "ReduceScatter", mybir.AluOpType.add,
                        replica_groups=[[0, 1], [2, 3], [4, 5], [6, 7]],
                        ins=[a2a_in[ci].opt()], outs=[a2a_out[ci].opt()],
                    )

            for mp in range(4):
                qs, ks = qkT[mp], qkT[4 + mp]
                for J in range(4):
                    _flush_pending(4 * mp + J - 1)
                    # interleave the NEXT head-pair's qk projection into this
                    # (ACT-bound) attention block: weights at J=0/2, two
                    # matmul groups per J.
                    if mp < 3:
                        if J == 0:
                            qk_load(mp + 1)
                        if J == 2:
                            qk_load(5 + mp)
                        qk_m = (mp + 1) if J < 2 else (5 + mp)
                        qk_ns = (2 * (J % 2), 2 * (J % 2) + 1)
                    if mp == 1 and J == 0:
                        # w_proj only needed at projection time; keep it off
                        # the phase-1 DMA queue
                        for hp in range(8):
                            nc.sync.dma_start(
                                wproj_t[hp][:],
                                wproj_d.ap()[hp * P : (hp + 1) * P, :],
                            )
                    if mp == 3 and J == 0:
                        # xT is dead once mp3's qk groups were emitted (end of
                        # mp2): prefetch the first two exchange chunks into the
                        # xT tiles while mp3's attention runs. On the gpsimd
                        # (SWDGE) queue: these wait on the collectives, and on
                        # the sync ring that wait would head-of-line-block the
                        # reciprocal chains behind it.
                        for h in range(6):
                            ci, r = (0, h) if h < 4 else (1, h - 4)
                            nc.gpsimd.dma_start(
                                xT[h][:, 0:1024],
                                a2a_out[ci][r * P : (r + 1) * P, :],
                            )
                    nj = 4 * J + 4
                    ouA = oups.tile([65, 512], f32, tag="ouA")
                    ouB = oups.tile([65, 512], f32, tag="ouB")
                    Js = slice(J * 512, (J + 1) * 512)
                    for j in range(nj):
                        sT = stps.tile([P, 1024], f32, tag="sT")
                        js = slice(j * P, (j + 1) * P)
                        nc.tensor.matmul(
                            sT[:, 0:512],
                            ks[0:64, js], qs[0:64, Js],
                            start=True, stop=True, tile_position=(0, 0),
                        )
                        nc.tensor.matmul(
                            sT[:, 512:1024],
                            ks[64:128, js], qs[64:128, Js],
                            start=True, stop=True, tile_position=(64, 0),
                        )
                        at = atp.tile([P, 1024], bf16, tag="at")
                        i = j - 4 * J
                        if i > 0:
                            c0 = 128 * i
                            src_v = sT[:].rearrange("p (h c) -> p h c", h=2)
                            dst_v = at[:].rearrange("p (h c) -> p h c", h=2)
                            nc.scalar.activation(
                                dst_v[:, :, c0:512], src_v[:, :, c0:512],
                                EXP, bias=0.0, scale=0.125,
                            )
                        else:
                            nc.scalar.activation(at[:], sT[:], EXP, bias=0.0, scale=0.125)
                        if i >= 0:
                            # diagonal-straddling block: zero cols < 128i, apply
                            # the triangle on cols [128i, 128i+128)
                            for h0 in (0, 512):
                                c0 = h0 + 128 * i
                                if i > 0:
                                    nc.vector.tensor_copy(
                                        at[:, h0 : h0 + 128 * i],
                                        zeros384[:, 0 : 128 * i],
                                    )
                                nc.vector.tensor_mul(
                                    at[:, c0 : c0 + 128],
                                    at[:, c0 : c0 + 128], mask_tri[:],
                                )
                        if mp == 0 and J == 0 and j == 0:
                            _dbg(nc, "at000", at[:])
                        nc.tensor.matmul(
                            ouA[:], vv[j][:, 130 * mp : 130 * mp + 65],
                            at[:, 0:512],
                            start=(j == 0), stop=(j == nj - 1),
                        )
                        nc.tensor.matmul(
                            ouB[:], vv[j][:, 130 * mp + 65 : 130 * mp + 130],
                            at[:, 512:1024],
                            start=(j == 0), stop=(j == nj - 1),
                        )
                        if mp < 3 and (j == nj // 2 - 1 or j == nj - 1):
                            qk_emit(qkps, qk_m, qk_ns[0 if j == nj // 2 - 1 else 1])
                    # normalize by softmax denominator (psum row 64) and evict
                    if mp == 0 and J == 0 and _DEBUG_SINK is not None:
                        for _nm, _ou in (("ouA00", ouA), ("ouB00", ouB)):
                            if _nm in _DEBUG_SINK:
                                _dt = atp.tile([65, 512], f32, tag=f"dbg{_nm}", name=f"dbg{_nm}")
                                nc.vector.tensor_copy(_dt[:], _ou[:])
                                nc.sync.dma_start(_DEBUG_SINK[_nm].ap(), _dt[:])
                    # Normalization stage 1: raw-evict o as bf16 (frees the
                    # psum banks fast), pull the denominator rows out, repack
                    # them onto 128 partitions via tiny SBUF-SBUF DMAs, and
                    # fast-reciprocal there. Stage 2 (PE-broadcast + multiply
                    # + ship to the exchange buffer) is deferred two J-slots
                    # so nothing ever waits on this chain.
                    dd = recip.tile([1, 1024], f32, tag="dd", name="dd")
                    rr = recip.tile([2, 512], f32, tag="rr", name="rr")
                    tb = tmpb.tile([64, 512], bf16, tag="tb")
                    nc.vector.tensor_copy(dd[0:1, 0:512], ouA[64:65, :])
                    nc.vector.tensor_copy(outN[mp][J][0:64, :], ouA[0:64, :])
                    nc.vector.tensor_copy(dd[0:1, 512:1024], ouB[64:65, :])
                    nc.vector.tensor_copy(tb[:], ouB[0:64, :])
                    # head B lives on partitions 64-127: DVE can't cross
                    # partitions, so DMA-shift the block up (ACT HWDGE ring;
                    # the sync ring carries the bulk loads).
                    nc.scalar.dma_start(outN[mp][J][64:128, :], tb[:])
                    # reciprocal on the single denominator row, then a DRAM
                    # hop to split it across partitions 0-1 for the broadcast
                    nc.vector.reciprocal_approx_fast(dd[:], dd[:])
                    dramD = dramDp.tile([2, 512], f32, tag="dramD", name="dramD")
                    nc.sync.dma_start(dramD[:].rearrange("a c -> (a c)").unsqueeze(0), dd[:])
                    nc.sync.dma_start(rr[:], dramD[:])
                    pending.append((4 * mp + J, mp, J, rr))
                    if mp == 0 and J == 0:
                        _dbg(nc, "outNraw00", outN[0][0][:])
                        _dbg(nc, "dd00", dd[:])

                if mp == CHUNKS[0][-1]:
                    _flush_pending(4 * mp + 3)
                    _emit_exchange(0)
                elif mp == CHUNKS[1][-1]:
                    _flush_pending(4 * mp + 3)
                    _emit_exchange(1)
            _flush_pending(15)
            _emit_exchange(2)
            _dbg(nc, "qkT0", qkT[0][:])
            _dbg(nc, "qkT4", qkT[4][:])

            # ---- output projection over my T-half, contracting all 16 heads.
            # po rows arrive in group-rank order == global head order on both
            # cores; wproj rows are host-permuted to match.
            # exchange landing buffers reuse the (dead) xT tiles; chunks 0-1
            # were prefetched at mp3 J0, only chunk 2 lands here.
            finp = p2.enter_context(tc.tile_pool(name="finp", bufs=6))
            po = list(xT)
            for h in (6, 7):
                nc.gpsimd.dma_start(
                    xT[h][:, 0:1024], a2a_out[2][(h - 6) * P : (h - 5) * P, :]
                )
            _dbg(nc, "po0", po[0][:, 0:1024])
            for i in range(8):
                for n in range(2):
                    u = i * 2 + n
                    pool_, tag_ = (qkps, "qkp") if u % 2 == 0 else (auxps, "aux")
                    ps = pool_.tile([P, 512], f32, tag=tag_, name="fp")
                    for hp in range(8):
                        nc.tensor.matmul(
                            ps[:],
                            po[hp][:, i * P : (i + 1) * P],
                            wproj_t[hp][:, n * 512 : (n + 1) * 512],
                            start=(hp == 0), stop=False,
                        )
                    # beta folded in as a rank-1 matmul so the eviction is a
                    # plain copy
                    nc.tensor.matmul(
                        ps[:], ones_row[:], beta_b[0:1, n * 512 : (n + 1) * 512],
                        start=False, stop=True,
                    )
                    fin = finp.tile([P, 512], f32, tag="fin", name="fin")
                    nc.vector.tensor_copy(fin[:], ps[:])
                    eng = nc.sync if u % 2 == 0 else nc.scalar
                    eng.dma_start(
                        out_d.ap()[i * P : (i + 1) * P, n * 512 : (n + 1) * 512], fin[:]
                    )


def _build():
    if "nc" in _CACHE:
        return _CACHE["nc"]
    global _DEBUG_SINK
    nc = bacc.Bacc("TRN2", target_bir_lowering=False, debug=False, num_devices=NCORES)
    spec = globals().get("_DEBUG_SINK_SPEC")
    if spec:
        _DEBUG_SINK = {
            name: nc.dram_tensor(name, list(shape), dt, kind="ExternalOutput")
            for name, (shape, dt) in spec.items()
        }
    x_d = nc.dram_tensor("x", [T, D], f32r, kind="ExternalInput")
    wqk_d = nc.dram_tensor("w_qk", [D, 1024], bf16, kind="ExternalInput")
    wv_d = nc.dram_tensor("w_v", [D, 512], bf16, kind="ExternalInput")
    bqk_d = nc.dram_tensor("b_qk", [8, P, 1], f32, kind="ExternalInput")
    wproj_d = nc.dram_tensor("w_proj", [D, D], bf16, kind="ExternalInput")
    beta_d = nc.dram_tensor("beta", [1, D], bf16, kind="ExternalInput")
    selab_d = nc.dram_tensor("selab", [2, 2, P], f32r, kind="ExternalInput")
    out_d = nc.dram_tensor("out", [T // 2, D], f32, kind="ExternalOutput")
    with tile.TileContext(nc) as tc:
        _emit(nc, tc, x_d, wqk_d, wv_d, bqk_d, wproj_d, beta_d, selab_d, out_d)
    nc.compile()
    _CACHE["nc"] = nc
    return nc


def make_in_maps(x, w_qkv, b_qkv, w_proj, b_proj):
    x = np.asarray(x, np.float32)
    w_qkv = np.asarray(w_qkv, np.float32)
    b_qkv = np.asarray(b_qkv, np.float32)
    w_proj = np.asarray(w_proj, np.float32)
    b_proj = np.asarray(b_proj, np.float32)
    # w_proj rows permuted into the exchange arrival order: for chunk c and
    # group rank gr, the head-pairs (mp, 4+mp) for mp in {2c, 2c+1} of core gr.
    # head-pair mp holds heads (2mp, 2mp+1) of the core's 8 (qkT/vv layout).
    # Chunk layout must match CHUNKS in _emit.
    perm = []
    for mps in ((0, 1), (2,), (3,)):
        for gr in range(2):
            for mp in mps:
                for m in (2 * mp, 2 * mp + 1):
                    h = gr * 8 + m
                    perm.extend(range(h * HD, (h + 1) * HD))
    wp_perm = np.ascontiguousarray(w_proj[perm, :]).astype(ml_dtypes.bfloat16)
    beta = (b_proj + w_proj.T @ b_qkv[2 * D :]).reshape(1, D).astype(ml_dtypes.bfloat16)
    # selector pattern: row 0 -> partitions 0-63 (head A), row 1 -> 64-127.
    selpat = np.zeros((2, P), np.float32)
    selpat[0, 0:64] = 1.0
    selpat[1, 64:128] = 1.0
    selz = np.zeros((2, P), np.float32)
    in_maps = []
    for c in range(NCORES):
        b, g = c // 2, c % 2
        qcols = slice(g * 512, (g + 1) * 512)
        kcols = slice(D + g * 512, D + (g + 1) * 512)
        vcols = slice(2 * D + g * 512, 2 * D + (g + 1) * 512)
        w_qk = np.concatenate([w_qkv[:, qcols], w_qkv[:, kcols]], axis=1)
        b_qk = np.concatenate([b_qkv[qcols], b_qkv[kcols]])
        in_maps.append({
            "x": np.ascontiguousarray(x[b]),
            "w_qk": np.ascontiguousarray(w_qk).astype(ml_dtypes.bfloat16),
            "w_v": np.ascontiguousarray(w_qkv[:, vcols]).astype(ml_dtypes.bfloat16),
            "b_qk": b_qk.reshape(8, P, 1),
            "w_proj": wp_perm,
            "beta": beta,
            "selab": np.stack([selpat, selz] if g == 0 else [selz, selpat]),
        })
    return in_maps


def kernel(x, w_qkv, b_qkv, w_proj, b_proj, trace=False, **run_kwargs):
    global LAST_RESULTS
    nc = _build()
    in_maps = make_in_maps(x, w_qkv, b_qkv, w_proj, b_proj)
    res = run_bass_kernel_spmd(
        nc, in_maps, core_ids=list(range(NCORES)), trace=trace, **run_kwargs
    )
    LAST_RESULTS = res
    out = np.empty((B, T, D), np.float32)
    for b in range(B):
        out[b, : T // 2] = res.results[2 * b]["out"]
        out[b, T // 2 :] = res.results[2 * b + 1]["out"]
    return out



# revision 66
# speedup vs baseline: 1.0373x; 1.0066x over previous
"""Causal self-attention Bass kernel for 8 trn2 NeuronCores.

Problem: B=4, T=2048, D=1024, H=16 causal self-attention (qkv proj + attn + out proj).

Sharding: core c = 2*b + g handles batch b (=c//2) and head-group g (=c%2, 8 heads).
Per core:
  - qkv projection column-shard: q,k,v columns for its 8 heads only. The q/k
    matmul groups for head-pair mp+1 are interleaved into the (ACT-bound)
    attention loop of head-pair mp so the PE stays dense and HAM-warm; mp0's
    ride the phase-1 transpose/v pipeline.
  - flash-style attention in transposed-score layout sT[tk, tq]; softmax
    denominator via an extra ones-column in the AV matmul (row 64 of the
    [65, 512] psum output). Normalization: fast-approx reciprocal of the
    denominator row, partition-broadcast via a tiny PE matmul (sel.T @ rr),
    deferred one J-slot so nothing waits on the chain.
  - instead of projecting partials and ReduceScattering [T, D] f32, the
    normalized per-head outputs o (bf16, 4x less data) are exchanged between
    the two cores of a batch with pairwise ReduceScatters: each core writes
    its o into both the even- and odd-core row slots of the exchange buffer,
    with the peer's slot zeroed via a per-core 0/1 selector, so the RS sum
    concatenates the heads. 3 chunks (mp01 / mp2 / mp3) so only the last
    1MB chunk is tail-exposed.
  - each core then projects its T-half contracting all 16 heads (w_proj rows
    host-permuted into exchange arrival order, beta folded in as a rank-1
    matmul), writing out rows [0,1024) (even core) / [1024,2048) (odd).
Host reassembles by stacking the two halves per batch.

Precision: matmuls bf16 (x, q/k, attn weights, v, o, w_proj) with f32 psum;
softmax is shift-robust and the 2e-2 rel-err budget absorbs bf16 rounding
(measured ~3.4e-3). b_v is folded into beta = b_proj + w_proj.T @ b_v since
softmax rows sum to 1.
"""

from contextlib import ExitStack

import ml_dtypes
import numpy as np

import concourse.bass as bass
import concourse.mybir as mybir
import concourse.tile as tile
from concourse import bacc
from concourse.bass_utils import run_bass_kernel_spmd

B, T, D, H = 4, 2048, 1024, 16
HD = D // H  # 64
NCORES = 8
P = 128
f32 = mybir.dt.float32
f32r = mybir.dt.float32r
bf16 = mybir.dt.bfloat16
EXP = mybir.ActivationFunctionType.Exp
LN = mybir.ActivationFunctionType.Ln

_CACHE = {}
LAST_RESULTS = None
_DEBUG_SINK = None


def _dbg(nc, name, ap):
    if _DEBUG_SINK is not None and name in _DEBUG_SINK:
        nc.sync.dma_start(_DEBUG_SINK[name].ap(), ap)


def _emit(nc, tc, x_d, wqk_d, wv_d, bqk_d, wproj_d, beta_d, selab_d, out_d):
    with ExitStack() as ctx:
        # ---------------- constants / persistent tiles ----------------
        const = ctx.enter_context(tc.tile_pool(name="const", bufs=1))
        bootc = ctx.enter_context(tc.tile_pool(name="boot", bufs=1))
        ident_f = bootc.tile([P, P], bf16, tag="ident_f")
        nc.gpsimd.memset(ident_f[:], 0.0)
        nc.gpsimd.affine_select(
            out=ident_f[:], in_=ident_f[:],
            compare_op=mybir.AluOpType.not_equal, fill=1.0,
            base=0, pattern=[[-1, P]], channel_multiplier=1,
        )
        ident = const.tile([P, P], f32r, tag="ident")
        nc.vector.tensor_copy(ident[:], ident_f[:])
        # (boot tiles stay resident; ~2.5KB)
        # triangle mask [128,128]: keep (1.0) iff f >= p
        mask_tri = const.tile([P, P], bf16, tag="mask_tri")
        nc.gpsimd.memset(mask_tri[:], 1.0)
        nc.gpsimd.affine_select(
            out=mask_tri[:], in_=mask_tri[:],
            compare_op=mybir.AluOpType.is_ge, fill=0.0,
            base=0, pattern=[[1, P]], channel_multiplier=-1,
        )
        bq = [const.tile([P, 1], f32, tag=f"bq{m}", name=f"bq{m}") for m in range(8)]
        beta_b = const.tile([P, D], bf16, tag="beta_b")
        # Per-core selectors for the denominator-reciprocal partition
        # broadcast: bc = sel.T @ rr puts rr row 0 on partitions 0-63, row 1
        # on 64-127. selE is the live pattern on even cores and all-zero on
        # odd cores (vice versa for selO) — this zeroes the exchange slot
        # that belongs to the peer, making the ReduceScatter sum a concat.
        selE = const.tile([2, P], f32r, tag="selE")
        selO = const.tile([2, P], f32r, tag="selO")

        def _load_small_consts():
            for m in range(8):
                nc.sync.dma_start(bq[m][:], bqk_d.ap()[m])
            nc.sync.dma_start(beta_b[0:1, :], beta_d.ap())
            nc.gpsimd.partition_broadcast(beta_b[:], beta_b[0:1, :], channels=P)
            nc.sync.dma_start(selE[:], selab_d.ap()[0])
            nc.sync.dma_start(selO[:], selab_d.ap()[1])
        # w_proj pool reserved here; its DMAs are emitted after phase 1 starts
        # so the x loads win the DMA queue.
        wpp = ctx.enter_context(tc.tile_pool(name="wpp", bufs=1))
        wproj_t = [wpp.tile([P, D], bf16, tag=f"wp{hp}", name=f"wp{hp}") for hp in range(8)]
        _dbg(nc, "beta_b", beta_b[:])

        # persistent activations
        xt_pool = ctx.enter_context(tc.tile_pool(name="xt", bufs=1))
        xT = [xt_pool.tile([P, T], bf16, tag=f"xT{k}", name=f"xT{k}") for k in range(8)]
        vv_pool = ctx.enter_context(tc.tile_pool(name="vv", bufs=1))
        vv = [vv_pool.tile([P, 520], bf16, tag=f"vv{i}", name=f"vv{i}") for i in range(16)]
        on_pool = ctx.enter_context(tc.tile_pool(name="outn", bufs=1))
        outN = [[on_pool.tile([P, 512], bf16, tag=f"outN{mp}J{J}", name=f"outN{mp}J{J}")
                 for J in range(4)] for mp in range(4)]
        zeros384 = const.tile([P, 384], bf16, tag="zeros384")
        nc.vector.memset(zeros384[:], 0.0)
        ones8 = const.tile([P, 8], bf16, tag="ones8")
        nc.vector.memset(ones8[:], 1.0)
        ones_row = const.tile([1, P], bf16, tag="ones_row")
        nc.vector.memset(ones_row[:], 1.0)
        ones_src = ones8[:].rearrange("p (mp h one) -> p mp h one", mp=4, h=2)
        for i in range(16):
            dst = vv[i][:].rearrange("p (mp h d) -> p mp h d", mp=4, h=2)
            nc.vector.tensor_copy(dst[:, :, :, 64:65], ones_src[:, :, :, :])

        # Head-half exchange buffers, chunked so only the last (single
        # head-pair) chunk is tail-exposed. In-tile rows per chunk (L pairs):
        # [shard s (T-col half) x [even-core slot (128L), odd-core slot
        # (128L)]]; each core writes BOTH slots, with the peer's slot zeroed
        # via selE/selO, so the pairwise ReduceScatter sum concatenates the
        # two cores' heads. Out rows arrive in a fixed global order on both
        # cores; wproj rows are host-permuted to match.
        CHUNKS = [(0, 1), (2,), (3,)]
        CHUNK_OF = {mp: ci for ci, mps in enumerate(CHUNKS) for mp in mps}
        dram = ctx.enter_context(tc.tile_pool(name="dram", bufs=1, space="DRAM"))
        a2a_in = [
            dram.tile([512 * len(mps), 1024], bf16, tag=f"a2a_in{c}",
                      name=f"a2a_in{c}")
            for c, mps in enumerate(CHUNKS)
        ]
        a2a_out = [
            dram.tile([256 * len(mps), 1024], bf16, tag=f"a2a_out{c}",
                      name=f"a2a_out{c}")
            for c, mps in enumerate(CHUNKS)
        ]

        # qk projection state (filled incrementally, interleaved into the
        # ACT-bound attention loop to keep the PE warm)
        qkt_pool = ctx.enter_context(tc.tile_pool(name="qkt", bufs=1))
        qkT = [qkt_pool.tile([P, T], bf16, tag=f"qkT{m}", name=f"qkT{m}") for m in range(8)]
        wqkp = ctx.enter_context(tc.tile_pool(name="wqk", bufs=2))
        wq_tiles = {}

        def qk_load(m):
            tiles = []
            for k in range(8):
                wt = wqkp.tile([P, P], bf16, tag=f"wqkt{k}", name=f"wqkt{k}")
                nc.sync.dma_start(
                    wt[:], wqk_d.ap()[k * P : (k + 1) * P, m * P : (m + 1) * P]
                )
                tiles.append(wt)
            wq_tiles[m] = tiles

        def qk_emit(pool, m, n):
            ps = pool.tile([P, 512], f32, tag="qkp", name="qkp")
            for k in range(8):
                nc.tensor.matmul(
                    ps[:], wq_tiles[m][k][:],
                    xT[k][:, n * 512 : (n + 1) * 512],
                    start=(k == 0), stop=(k == 7),
                )
            nc.vector.tensor_scalar_add(
                qkT[m][:, n * 512 : (n + 1) * 512], ps[:], bq[m][:]
            )

        # ---------------- phase 1: load x, transpose, compute v ----------------
        with ExitStack() as p1:
            xload = p1.enter_context(tc.tile_pool(name="xload", bufs=5))
            wvp = p1.enter_context(tc.tile_pool(name="wv", bufs=1))
            tpps = p1.enter_context(tc.tile_pool(name="tpps", bufs=2, space="PSUM"))
            vps = p1.enter_context(tc.tile_pool(name="vps", bufs=2, space="PSUM"))
            wv_t = [wvp.tile([P, 512], bf16, tag=f"wvt{k}", name=f"wvt{k}") for k in range(8)]
            for qq in range(4):  # t-quarters
                xi = []
                for ii in range(4):
                    xt_ = xload.tile([P, D], f32r, tag="x")
                    r0 = (qq * 4 + ii) * P
                    nc.sync.dma_start(xt_[:], x_d.ap()[r0 : r0 + P, :])
                    xi.append(xt_)
                if qq == 0:
                    # weight loads queue after the first x tiles
                    for k in range(8):
                        nc.sync.dma_start(wv_t[k][:], wv_d.ap()[k * P : (k + 1) * P, :])
                    qk_load(0)
                    qk_load(4)
                    _load_small_consts()
                for k in range(8):
                    tp = tpps.tile([P, 512], f32r, tag="tp")
                    for ii in range(4):
                        nc.tensor.transpose(
                            tp[:, ii * P : (ii + 1) * P],
                            xi[ii][:, k * P : (k + 1) * P],
                            ident[:],
                        )
                    nc.scalar.copy(xT[k][:, qq * 512 : (qq + 1) * 512], tp[:])
                # v for this quarter's 4 t-tiles (bf16 matmul off resident xT)
                for il in range(4):
                    i = qq * 4 + il
                    ps = vps.tile([P, 512], f32, tag="vp")
                    for k in range(8):
                        nc.tensor.matmul(
                            ps[:],
                            xT[k][:, (qq * 4 + il) * P : (qq * 4 + il + 1) * P],
                            wv_t[k][:],
                            start=(k == 0), stop=(k == 7),
                        )
                    # strided evict: psum [p, (mp h d)] d=64 -> vv [p, (mp h d65)]
                    src = ps[:].rearrange("p (mp h d) -> p mp h d", mp=4, h=2)
                    dst = vv[i][:].rearrange("p (mp h d) -> p mp h d", mp=4, h=2)
                    nc.vector.tensor_copy(dst[:, :, :, 0:64], src[:, :, :, :])
                # mp0's qk projection for this quarter rides the transpose/v
                # phase (xT quarter just landed)
                qk_emit(vps, 0, qq)
                qk_emit(vps, 4, qq)
            _dbg(nc, "xT0", xT[0][:])
            _dbg(nc, "vv0", vv[0][:])

        # ---------------- phase 2: per head-pair qkv + attention ----------------
        with ExitStack() as p2:
            atp = p2.enter_context(tc.tile_pool(name="atp", bufs=3))
            recip = p2.enter_context(tc.tile_pool(name="recip", bufs=4))
            shipp = p2.enter_context(tc.tile_pool(name="shipp", bufs=4))
            tmpb = p2.enter_context(tc.tile_pool(name="tmpb", bufs=2))
            dramDp = p2.enter_context(tc.tile_pool(name="dramDp", bufs=4, space="DRAM"))
            qkps = p2.enter_context(tc.tile_pool(name="qkps", bufs=1, space="PSUM"))
            stps = p2.enter_context(tc.tile_pool(name="stps", bufs=2, space="PSUM"))
            oups = p2.enter_context(tc.tile_pool(name="oups", bufs=1, space="PSUM"))
            auxps = p2.enter_context(tc.tile_pool(name="auxps", bufs=1, space="PSUM"))

            # deferred normalization stage-2: (mp, J, rr) emitted ~2 J-slots
            # later so the PE-broadcast matmul never waits on the reciprocal.
            pending = []

            def _norm_stage2(mp_, J_, rr_):
                c0 = (J_ % 2) * 512
                ci = CHUNK_OF[mp_]
                L = len(CHUNKS[ci])
                idx = mp_ - CHUNKS[ci][0]
                for sslot, sel in enumerate((selE, selO)):
                    bc = auxps.tile([P, 512], f32, tag="aux", name="bc")
                    nc.tensor.matmul(
                        bc[:], sel[:], rr_[:].bitcast(f32r), start=True, stop=True
                    )
                    tmp = shipp.tile([P, 512], bf16, tag="ship", name="ship")
                    nc.vector.tensor_mul(tmp[:], outN[mp_][J_][:], bc[:])
                    r0 = (J_ // 2) * 256 * L + sslot * 128 * L + idx * 128
                    nc.scalar.dma_start(
                        a2a_in[ci][r0 : r0 + P, c0 : c0 + 512], tmp[:]
                    )

            def _flush_pending(upto_slot):
                while pending and pending[0][0] <= upto_slot:
                    _, mp_, J_, rr_ = pending.pop(0)
                    _norm_stage2(mp_, J_, rr_)

            def _emit_exchange(ci):
                half = 256 * len(CHUNKS[ci])
                if globals().get("_NO_COLLECTIVE"):
                    nc.sync.dma_start(a2a_out[ci].opt(), a2a_in[ci][0:half, :])
                else:
                    nc.gpsimd.collective_compute(
                        "ReduceScatter", mybir.AluOpType.add,
                        replica_groups=[[0, 1], [2, 3], [4, 5], [6, 7]],
                        ins=[a2a_in[ci].opt()], outs=[a2a_out[ci].opt()],
                    )

            for mp in range(4):
                qs, ks = qkT[mp], qkT[4 + mp]
                for J in range(4):
                    _flush_pending(4 * mp + J - 1)
                    # interleave the NEXT head-pair's qk projection into this
                    # (ACT-bound) attention block: weights at J=0/2, two
                    # matmul groups per J.
                    if mp < 3:
                        if J == 0:
                            qk_load(mp + 1)
                        if J == 2:
                            qk_load(5 + mp)
                        qk_m = (mp + 1) if J < 2 else (5 + mp)
                        qk_ns = (2 * (J % 2), 2 * (J % 2) + 1)
                    if mp == 1 and J == 0:
                        # w_proj only needed at projection time; keep it off
                        # the phase-1 DMA queue
                        for hp in range(8):
                            nc.sync.dma_start(
                                wproj_t[hp][:],
                                wproj_d.ap()[hp * P : (hp + 1) * P, :],
                            )
                    if mp == 3 and J == 0:
                        # xT is dead once mp3's qk groups were emitted (end of
                        # mp2): prefetch the first two exchange chunks into the
                        # xT tiles while mp3's attention runs. On the gpsimd
                        # (SWDGE) queue: these wait on the collectives, and on
                        # the sync ring that wait would head-of-line-block the
                        # reciprocal chains behind it.
                        for h in range(6):
                            ci, r = (0, h) if h < 4 else (1, h - 4)
                            nc.gpsimd.dma_start(
                                xT[h][:, 0:1024],
                                a2a_out[ci][r * P : (r + 1) * P, :],
                            )
                    nj = 4 * J + 4
                    ouA = oups.tile([65, 512], f32, tag="ouA")
                    ouB = oups.tile([65, 512], f32, tag="ouB")
                    Js = slice(J * 512, (J + 1) * 512)
                    for j in range(nj):
                        sT = stps.tile([P, 1024], f32, tag="sT")
                        js = slice(j * P, (j + 1) * P)
                        nc.tensor.matmul(
                            sT[:, 0:512],
                            ks[0:64, js], qs[0:64, Js],
                            start=True, stop=True, tile_position=(0, 0),
                        )
                        nc.tensor.matmul(
                            sT[:, 512:1024],
                            ks[64:128, js], qs[64:128, Js],
                            start=True, stop=True, tile_position=(64, 0),
                        )
                        at = atp.tile([P, 1024], bf16, tag="at")
                        i = j - 4 * J
                        if i > 0:
                            c0 = 128 * i
                            src_v = sT[:].rearrange("p (h c) -> p h c", h=2)
                            dst_v = at[:].rearrange("p (h c) -> p h c", h=2)
                            nc.scalar.activation(
                                dst_v[:, :, c0:512], src_v[:, :, c0:512],
                                EXP, bias=0.0, scale=0.125,
                            )
                        else:
                            nc.scalar.activation(at[:], sT[:], EXP, bias=0.0, scale=0.125)
                        if i >= 0:
                            # diagonal-straddling block: zero cols < 128i, apply
                            # the triangle on cols [128i, 128i+128)
                            for h0 in (0, 512):
                                c0 = h0 + 128 * i
                                if i > 0:
                                    nc.vector.tensor_copy(
                                        at[:, h0 : h0 + 128 * i],
                                        zeros384[:, 0 : 128 * i],
                                    )
                                nc.vector.tensor_mul(
                                    at[:, c0 : c0 + 128],
                                    at[:, c0 : c0 + 128], mask_tri[:],
                                )
                        if mp == 0 and J == 0 and j == 0:
                            _dbg(nc, "at000", at[:])
                        nc.tensor.matmul(
                            ouA[:], vv[j][:, 130 * mp : 130 * mp + 65],
                            at[:, 0:512],
                            start=(j == 0), stop=(j == nj - 1),
                        )
                        nc.tensor.matmul(
                            ouB[:], vv[j][:, 130 * mp + 65 : 130 * mp + 130],
                            at[:, 512:1024],
                            start=(j == 0), stop=(j == nj - 1),
                        )
                        if mp < 3 and (j == nj // 2 - 1 or j == nj - 1):
                            qk_emit(qkps, qk_m, qk_ns[0 if j == nj // 2 - 1 else 1])
                    # normalize by softmax denominator (psum row 64) and evict
                    if mp == 0 and J == 0 and _DEBUG_SINK is not None:
                        for _nm, _ou in (("ouA00", ouA), ("ouB00", ouB)):
                            if _nm in _DEBUG_SINK:
                                _dt = atp.tile([65, 512], f32, tag=f"dbg{_nm}", name=f"dbg{_nm}")
                                nc.vector.tensor_copy(_dt[:], _ou[:])
                                nc.sync.dma_start(_DEBUG_SINK[_nm].ap(), _dt[:])
                    # Normalization stage 1: raw-evict o as bf16 (frees the
                    # psum banks fast), pull the denominator rows out, repack
                    # them onto 128 partitions via tiny SBUF-SBUF DMAs, and
                    # fast-reciprocal there. Stage 2 (PE-broadcast + multiply
                    # + ship to the exchange buffer) is deferred two J-slots
                    # so nothing ever waits on this chain.
                    dd = recip.tile([1, 1024], f32, tag="dd", name="dd")
                    rr = recip.tile([2, 512], f32, tag="rr", name="rr")
                    tb = tmpb.tile([64, 512], bf16, tag="tb")
                    nc.vector.tensor_copy(dd[0:1, 0:512], ouA[64:65, :])
                    nc.vector.tensor_copy(outN[mp][J][0:64, :], ouA[0:64, :])
                    nc.vector.tensor_copy(dd[0:1, 512:1024], ouB[64:65, :])
                    nc.vector.tensor_copy(tb[:], ouB[0:64, :])
                    # head B lives on partitions 64-127: DVE can't cross
                    # partitions, so DMA-shift the block up (ACT HWDGE ring;
                    # the sync ring carries the bulk loads).
                    nc.scalar.dma_start(outN[mp][J][64:128, :], tb[:])
                    # reciprocal on the single denominator row, then a DRAM
                    # hop to split it across partitions 0-1 for the broadcast
                    nc.vector.reciprocal_approx_fast(dd[:], dd[:])
                    dramD = dramDp.tile([2, 512], f32, tag="dramD", name="dramD")
                    nc.sync.dma_start(dramD[:].rearrange("a c -> (a c)").unsqueeze(0), dd[:])
                    nc.sync.dma_start(rr[:], dramD[:])
                    pending.append((4 * mp + J, mp, J, rr))
                    if mp == 0 and J == 0:
                        _dbg(nc, "outNraw00", outN[0][0][:])
                        _dbg(nc, "dd00", dd[:])

                if mp == CHUNKS[0][-1]:
                    _flush_pending(4 * mp + 3)
                    _emit_exchange(0)
                elif mp == CHUNKS[1][-1]:
                    _flush_pending(4 * mp + 3)
                    _emit_exchange(1)
            _flush_pending(15)
            _emit_exchange(2)
            _dbg(nc, "qkT0", qkT[0][:])
            _dbg(nc, "qkT4", qkT[4][:])

            # ---- output projection over my T-half, contracting all 16 heads.
            # po rows arrive in group-rank order == global head order on both
            # cores; wproj rows are host-permuted to match.
            # exchange landing buffers reuse the (dead) xT tiles; chunks 0-1
            # were prefetched at mp3 J0, only chunk 2 lands here.
            finp = p2.enter_context(tc.tile_pool(name="finp", bufs=3))
            po = list(xT)
            for h in (6, 7):
                nc.gpsimd.dma_start(
                    xT[h][:, 0:1024], a2a_out[2][(h - 6) * P : (h - 5) * P, :]
                )
            _dbg(nc, "po0", po[0][:, 0:1024])
            # one [128,1024] psum unit per t-tile, reusing the (now idle) sT
            # pool: fewer, larger evictions and 2-deep psum pipelining. hp 6/7
            # (the cc2-dependent chunk) accumulate last so hp 0-5 can start
            # while the final exchange is still in flight.
            for i in range(8):
                ps = stps.tile([P, 1024], f32, tag="sT", name="fp")
                for n in range(2):
                    pn = ps[:, n * 512 : (n + 1) * 512]
                    for hp in range(8):
                        nc.tensor.matmul(
                            pn,
                            po[hp][:, i * P : (i + 1) * P],
                            wproj_t[hp][:, n * 512 : (n + 1) * 512],
                            start=(hp == 0), stop=False,
                        )
                    # beta folded in as a rank-1 matmul so the eviction is a
                    # plain copy
                    nc.tensor.matmul(
                        pn, ones_row[:], beta_b[0:1, n * 512 : (n + 1) * 512],
                        start=False, stop=True,
                    )
                fin = finp.tile([P, 1024], f32, tag="fin", name="fin")
                nc.vector.tensor_copy(fin[:], ps[:])
                eng = nc.sync if i % 2 == 0 else nc.scalar
                eng.dma_start(out_d.ap()[i * P : (i + 1) * P, :], fin[:])


def _build():
    if "nc" in _CACHE:
        return _CACHE["nc"]
    global _DEBUG_SINK
    nc = bacc.Bacc("TRN2", target_bir_lowering=False, debug=False, num_devices=NCORES)
    spec = globals().get("_DEBUG_SINK_SPEC")
    if spec:
        _DEBUG_SINK = {
            name: nc.dram_tensor(name, list(shape), dt, kind="ExternalOutput")
            for name, (shape, dt) in spec.items()
        }
    x_d = nc.dram_tensor("x", [T, D], f32r, kind="ExternalInput")
    wqk_d = nc.dram_tensor("w_qk", [D, 1024], bf16, kind="ExternalInput")
    wv_d = nc.dram_tensor("w_v", [D, 512], bf16, kind="ExternalInput")
    bqk_d = nc.dram_tensor("b_qk", [8, P, 1], f32, kind="ExternalInput")
    wproj_d = nc.dram_tensor("w_proj", [D, D], bf16, kind="ExternalInput")
    beta_d = nc.dram_tensor("beta", [1, D], bf16, kind="ExternalInput")
    selab_d = nc.dram_tensor("selab", [2, 2, P], f32r, kind="ExternalInput")
    out_d = nc.dram_tensor("out", [T // 2, D], f32, kind="ExternalOutput")
    with tile.TileContext(nc) as tc:
        _emit(nc, tc, x_d, wqk_d, wv_d, bqk_d, wproj_d, beta_d, selab_d, out_d)
    nc.compile()
    _CACHE["nc"] = nc
    return nc


def make_in_maps(x, w_qkv, b_qkv, w_proj, b_proj):
    x = np.asarray(x, np.float32)
    w_qkv = np.asarray(w_qkv, np.float32)
    b_qkv = np.asarray(b_qkv, np.float32)
    w_proj = np.asarray(w_proj, np.float32)
    b_proj = np.asarray(b_proj, np.float32)
    # w_proj rows permuted into the exchange arrival order: for chunk c and
    # group rank gr, the head-pairs (mp, 4+mp) for mp in {2c, 2c+1} of core gr.
    # head-pair mp holds heads (2mp, 2mp+1) of the core's 8 (qkT/vv layout).
    # Chunk layout must match CHUNKS in _emit.
    perm = []
    for mps in ((0, 1), (2,), (3,)):
        for gr in range(2):
            for mp in mps:
                for m in (2 * mp, 2 * mp + 1):
                    h = gr * 8 + m
                    perm.extend(range(h * HD, (h + 1) * HD))
    wp_perm = np.ascontiguousarray(w_proj[perm, :]).astype(ml_dtypes.bfloat16)
    beta = (b_proj + w_proj.T @ b_qkv[2 * D :]).reshape(1, D).astype(ml_dtypes.bfloat16)
    # selector pattern: row 0 -> partitions 0-63 (head A), row 1 -> 64-127.
    selpat = np.zeros((2, P), np.float32)
    selpat[0, 0:64] = 1.0
    selpat[1, 64:128] = 1.0
    selz = np.zeros((2, P), np.float32)
    in_maps = []
    for c in range(NCORES):
        b, g = c // 2, c % 2
        qcols = slice(g * 512, (g + 1) * 512)
        kcols = slice(D + g * 512, D + (g + 1) * 512)
        vcols = slice(2 * D + g * 512, 2 * D + (g + 1) * 512)
        w_qk = np.concatenate([w_qkv[:, qcols], w_qkv[:, kcols]], axis=1)
        b_qk = np.concatenate([b_qkv[qcols], b_qkv[kcols]])
        in_maps.append({
            "x": np.ascontiguousarray(x[b]),
            "w_qk": np.ascontiguousarray(w_qk).astype(ml_dtypes.bfloat16),
            "w_v": np.ascontiguousarray(w_qkv[:, vcols]).astype(ml_dtypes.bfloat16),
            "b_qk": b_qk.reshape(8, P, 1),
            "w_proj": wp_perm,
            "beta": beta,
            "selab": np.stack([selpat, selz] if g == 0 else [selz, selpat]),
        })
    return in_maps


def kernel(x, w_qkv, b_qkv, w_proj, b_proj, trace=False, **run_kwargs):
    global LAST_RESULTS
    nc = _build()
    in_maps = make_in_maps(x, w_qkv, b_qkv, w_proj, b_proj)
    res = run_bass_kernel_spmd(
        nc, in_maps, core_ids=list(range(NCORES)), trace=trace, **run_kwargs
    )
    LAST_RESULTS = res
    out = np.empty((B, T, D), np.float32)
    for b in range(B):
        out[b, : T // 2] = res.results[2 * b]["out"]
        out[b, T // 2 :] = res.results[2 * b + 1]["out"]
    return out

